# revision 20
# baseline (speedup 1.0000x reference)
"""BitLinear (2-bit ternary packed weights) Trainium2 Bass kernel.

Full-input contract: kernel(x, weight, weight_scale) -> (2, 2048, 12288) f32.
Tensor-parallel over 8 NeuronCores: weight rows (out_features) sharded
8 x 1536, x replicated, outputs concatenated host-side.

Math notes:
  - reference: x_i8 = round(x * 127/absmax_tok); W = unpack2bit(weight)-1
    out = (x_i8 @ W.T) * weight_scale * absmax_tok/127
  - we matmul against the raw 2-bit codes c in {0,1,2,3} (bf16-exact) and
    fold the -1 via out_int = psum - rowsum(x_i8); rowsum comes free via
    accum_out on the quantize op.
  - bf16 holds integers up to 256 exactly; f32 PSUM accumulation of
    integer products stays < 2^24 => matmul is bit-exact vs f32 reference.
  - rounding uses the +/- 1.5*2^23 magic trick == round-half-to-even
    (matches jnp.round). clip(-128,127) is a provable no-op since
    |x*q| <= 127*(1+eps).

Layout: weights are unpacked+transposed once into per-(m-group, k-tile)
SBUF tiles WT[mg][t] ([128 k, 512 m] bf16) so each matmul depends on
exactly one prologue producer (fine-grained Tile deps -> early PE start).
"""

import os
from contextlib import ExitStack

import numpy as np

import concourse.bass as bass
import concourse.mybir as mybir
import concourse.tile as tile
from concourse import bacc
from concourse.bass import ds, ts
from concourse.bass_utils import run_bass_kernel_spmd
from concourse.masks import make_identity

# problem shapes (hardcoded per contract)
B, T, K, M = 2, 2048, 4096, 12288
N = B * T
N_CORES = 8
M_CORE = M // N_CORES

MAGIC = 12582912.0  # 1.5 * 2**23: add+sub forces RNE rounding to integer

f32 = mybir.dt.float32
bf16 = mybir.dt.bfloat16
i32 = mybir.dt.int32
Alu = mybir.AluOpType
Act = mybir.ActivationFunctionType
Ax = mybir.AxisListType


def declare_io(nc: bass.Bass, n: int, k: int, m_core: int):
    x = nc.dram_tensor("x", [n, k], f32, kind="ExternalInput").ap()
    wp = nc.dram_tensor("wp", [m_core, k // 4], i32, kind="ExternalInput").ap()
    ws = nc.dram_tensor("ws", [1], f32, kind="ExternalInput").ap()
    out = nc.dram_tensor("out", [n, m_core], f32, kind="ExternalOutput").ap()
    return x, wp, ws, out


def emit(tc: tile.TileContext, ctx: ExitStack, aps, n: int, k: int, m_core: int):
    nc = tc.nc
    x, wp, ws, out = aps
    assert n % 128 == 0 and k % 512 == 0 and m_core % 128 == 0
    KT = k // 128  # number of 128-wide k tiles (== number of packed 32B blocks)
    NT = n // 128
    MC = 512 if m_core % 512 == 0 else m_core
    assert m_core % MC == 0
    NMC = m_core // MC  # m groups (moving-operand chunks)
    MTPG = MC // 128  # m tiles per group
    KH = KT // 2 if KT % 2 == 0 else KT  # ktiles per weight staging chunk
    NH = KT // KH

    const = ctx.enter_context(tc.tile_pool(name="const", bufs=1))
    ident = const.tile([128, 128], bf16)
    make_identity(nc, ident[:])
    wsb = const.tile([128, 1], f32)
    nc.gpsimd.dma_start(out=wsb[:], in_=ws.to_broadcast((128, 1)))

    # per-(m-group, k-tile) weight tiles: [128 k, MC m] bf16 codes {0..3}
    wt_pool = ctx.enter_context(tc.tile_pool(name="wt", bufs=1))
    WT = [
        [
            wt_pool.tile(
                [128, MC], bf16, tag=f"wt{mg}_{t}", name=f"wt{mg}_{t}"
            )
            for t in range(KT)
        ]
        for mg in range(NMC)
    ]

    # ---------------- x quantization (DVE) ----------------
    xf_pool = ctx.enter_context(tc.tile_pool(name="xf", bufs=2))
    xq_pool = ctx.enter_context(tc.tile_pool(name="xq", bufs=2))
    sm_pool = ctx.enter_context(tc.tile_pool(name="sm", bufs=3))
    qstate = {}
    xfs = {}

    def xdma(ni):
        # split across SWDGE queues so the first token tile lands sooner
        xf = xf_pool.tile([128, k], f32, tag="xf", name=f"xf{ni}")
        half = k // 2
        nc.gpsimd.dma_start(out=xf[:, 0:half], in_=x[ts(ni, 128), 0:half])
        nc.gpsimd.dma_start(out=xf[:, half:k], in_=x[ts(ni, 128), half:k])
        xfs[ni] = xf

    def quant(ni):
        if ni not in xfs:
            xdma(ni)
        xf = xfs.pop(ni)
        am = sm_pool.tile([128, 1], f32)
        nc.vector.tensor_reduce(
            am[:], xf[:], axis=Ax.X, op=Alu.max, apply_absolute_value=True
        )
        am2 = sm_pool.tile([128, 1], f32)
        nc.vector.tensor_scalar_max(am2[:], am[:], 1e-5)
        r = sm_pool.tile([128, 1], f32)  # 1/absmax (HW iterative divide)
        nc.vector.reciprocal(r[:], am2[:])
        q = sm_pool.tile([128, 1], f32)  # 127/absmax (to ~1ulp of reference)
        nc.vector.tensor_scalar_mul(q[:], r[:], 127.0)
        a = sm_pool.tile([128, 1], f32)  # act_scale = 1/q
        nc.vector.reciprocal(a[:], q[:])
        s = sm_pool.tile([128, 1], f32)  # act_scale * weight_scale
        nc.vector.tensor_tensor(s[:], a[:], wsb[:], op=Alu.mult)

        # x_i8 = RNE(x*q): in-place (x*q + MAGIC), then (- MAGIC) -> bf16;
        # accum_out gives rowsum(x_i8) in the same pass.
        nc.vector.tensor_scalar(
            out=xf[:], in0=xf[:], scalar1=q[:], scalar2=MAGIC,
            op0=Alu.mult, op1=Alu.add,
        )
        xq = xq_pool.tile([128, k], bf16)
        rs = sm_pool.tile([128, 1], f32)
        # out = in - MAGIC; accum_out = reduce_add(out) (op1 names the reduce)
        nc.vector.tensor_scalar(
            out=xq[:], in0=xf[:], scalar1=MAGIC, scalar2=None,
            op0=Alu.subtract, op1=Alu.add, accum_out=rs[:],
        )
        nrss = sm_pool.tile([128, 1], f32)  # -rowsum * s  (bias for dequant)
        nc.vector.tensor_scalar(
            out=nrss[:], in0=rs[:], scalar1=s[:], scalar2=-1.0,
            op0=Alu.mult, op1=Alu.mult,
        )
        qstate[ni] = (xq, s, nrss)

    # start the first x loads now; their DVE compute is interleaved into the
    # weight prologue below (after the first chunk) so PE starts on W
    # transposes immediately while x data is still arriving.
    xdma(0)
    xdma(1)

    # ---------------- weight unpack prologue ----------------
    # packed byte b of row m (stored as one int32 each, value 0..255):
    # block = b//32, j = b%32; weight k = block*128 + g*32 + j uses shift 6-2g.
    pk_pool = ctx.enter_context(tc.tile_pool(name="pk", bufs=3))
    wi_pool = ctx.enter_context(tc.tile_pool(name="wi", bufs=2))
    wn_pool = ctx.enter_context(tc.tile_pool(name="wn", bufs=MTPG + 1))
    tpw_pool = ctx.enter_context(tc.tile_pool(name="tpw", bufs=2, space="PSUM"))
    kb_h = k // 4 // NH  # packed bytes per row per staging chunk

    for mg in range(NMC):
        for h in range(NH):
            wns = []
            for j in range(MTPG):
                mi = mg * MTPG + j
                pkt = pk_pool.tile([128, kb_h], i32)
                nc.gpsimd.dma_start(
                    out=pkt[:], in_=wp[ts(mi, 128), ds(h * kb_h, kb_h)]
                )
                pk_v = pkt[:].rearrange("p (b j) -> p b j", j=32)
                # bitVec ops cannot cast: extract int32, then convert to bf16.
                # g=3 (mask-only) runs on GpSimd; cast too — keeps ACT (which
                # suffers event-sem head-of-line blocking) out of this chain.
                wn_i = wi_pool.tile([128, KH, 4, 32], i32)  # [m, kt, g, j]
                for g in range(4):
                    nc.vector.tensor_scalar(
                        out=wn_i[:, :, g, :],
                        in0=pk_v,
                        scalar1=6 - 2 * g,
                        scalar2=3,
                        op0=Alu.logical_shift_right,
                        op1=Alu.bitwise_and,
                    )
                wn = wn_pool.tile([128, KH, 4, 32], bf16, tag="wn")
                nc.vector.tensor_copy(
                    wn[:].rearrange("p t g j -> p (t g j)"),
                    wn_i[:].rearrange("p t g j -> p (t g j)"),
                )
                wns.append(wn)
            for tq in range(KH):
                t = h * KH + tq
                tp = tpw_pool.tile([128, MC], bf16)
                for j in range(MTPG):
                    wn_v = wns[j][:].rearrange("p t g j -> p t (g j)")
                    nc.tensor.transpose(tp[:, ts(j, 128)], wn_v[:, tq, :], ident[:])
                nc.scalar.copy(out=WT[mg][t][:], in_=tp[:])
            if mg == 0:
                # interleave the first two token tiles' quantize chains
                if h == 0 and 0 not in qstate:
                    quant(0)
                if h == NH - 1 and 1 not in qstate:
                    quant(1)

    # ---------------- main loop over token tiles ----------------
    xt_pool = ctx.enter_context(tc.tile_pool(name="xt", bufs=2))
    ob_pool = ctx.enter_context(tc.tile_pool(name="ob", bufs=2))
    tpx_pool = ctx.enter_context(tc.tile_pool(name="tpx", bufs=2, space="PSUM"))
    mm_pool = ctx.enter_context(tc.tile_pool(name="mm", bufs=3, space="PSUM"))

    for ni in range(NT):
        if ni >= 2:
            quant(ni)
        xq, s, nrss = qstate.pop(ni)

        # transpose x_i8 -> xT [k%128, ktile, n%128]
        xT = xt_pool.tile([128, KT, 128], bf16)
        xq_v = xq[:].rearrange("p (t j) -> p t j", j=128)
        for tq in range((KT + 3) // 4):
            nt = min(4, KT - tq * 4)
            tp = tpx_pool.tile([128, 512], bf16)
            for j in range(nt):
                t = tq * 4 + j
                nc.tensor.transpose(tp[:, ts(j, 128)], xq_v[:, t, :], ident[:])
            nc.scalar.copy(
                out=xT[:, ds(tq * 4, nt), :],
                in_=tp[:, ds(0, nt * 128)].rearrange("p (a b) -> p a b", b=128),
            )

        for mc in range(NMC):
            ps = mm_pool.tile([128, MC], f32)
            for t in range(KT):
                nc.tensor.matmul(
                    ps[:],
                    lhsT=xT[:, t, :],
                    rhs=WT[mc][t][:],
                    start=(t == 0),
                    stop=(t == KT - 1),
                )
            ob = ob_pool.tile([128, MC], f32)
            # out = psum*s + (-rowsum*s)  == (psum - rowsum) * s
            nc.scalar.activation(
                ob[:], ps[:], Act.Identity, bias=nrss[:], scale=s[:]
            )
            nc.gpsimd.dma_start(out=out[ts(ni, 128), ds(mc * MC, MC)], in_=ob[:])


def build(n: int = N, k: int = K, m_core: int = M_CORE, num_devices: int = N_CORES):
    nc = bacc.Bacc(
        "TRN2", target_bir_lowering=False, debug=False, num_devices=num_devices
    )
    aps = declare_io(nc, n, k, m_core)
    with tile.TileContext(nc) as tc:
        with ExitStack() as ctx:
            emit(tc, ctx, aps, n, k, m_core)
    nc.compile()
    return nc


_CACHE: dict = {}


def kernel(x: np.ndarray, weight: np.ndarray, weight_scale: np.ndarray) -> np.ndarray:
    orig_shape = x.shape
    x2 = np.ascontiguousarray(x.reshape(-1, orig_shape[-1]), dtype=np.float32)
    w = np.ascontiguousarray(weight, dtype=np.int32)
    ws = np.ascontiguousarray(weight_scale, dtype=np.float32)
    assert x2.shape == (N, K) and w.shape == (M, K // 4)

    if "nc" not in _CACHE:
        _CACHE["nc"] = build()
    nc = _CACHE["nc"]

    in_maps = [
        {"x": x2, "wp": w[i * M_CORE : (i + 1) * M_CORE], "ws": ws}
        for i in range(N_CORES)
    ]
    res = run_bass_kernel_spmd(
        nc,
        in_maps,
        list(range(N_CORES)),
        trace=bool(int(os.environ.get("BITLINEAR_TRACE", "0"))),
    )
    _CACHE["last_result"] = res
    outs = [res.results[i]["out"] for i in range(N_CORES)]
    full = np.concatenate(outs, axis=1)
    return full.reshape(*orig_shape[:-1], M).astype(x.dtype, copy=False)


# revision 25
# speedup vs baseline: 1.0193x; 1.0193x over previous
"""BitLinear (2-bit ternary packed weights) Trainium2 Bass kernel.

Full-input contract: kernel(x, weight, weight_scale) -> (2, 2048, 12288) f32.
Tensor-parallel over 8 NeuronCores: weight rows (out_features) sharded
8 x 1536, x replicated, outputs concatenated host-side.

Math notes:
  - reference: x_i8 = round(x * 127/absmax_tok); W = unpack2bit(weight)-1
    out = (x_i8 @ W.T) * weight_scale * absmax_tok/127
  - we matmul against the raw 2-bit codes c in {0,1,2,3} (bf16-exact) and
    fold the -1 via out_int = psum - rowsum(x_i8); rowsum comes free via
    accum_out on the quantize op.
  - bf16 holds integers up to 256 exactly; f32 PSUM accumulation of
    integer products stays < 2^24 => matmul is bit-exact vs f32 reference.
  - rounding uses the +/- 1.5*2^23 magic trick == round-half-to-even
    (matches jnp.round). clip(-128,127) is a provable no-op since
    |x*q| <= 127*(1+eps).

Layout: weights are unpacked+transposed once into per-(m-group, k-tile)
SBUF tiles WT[mg][t] ([128 k, 512 m] bf16) so each matmul depends on
exactly one prologue producer (fine-grained Tile deps -> early PE start).
"""

import os
from contextlib import ExitStack

import numpy as np

import concourse.bass as bass
import concourse.mybir as mybir
import concourse.tile as tile
from concourse import bacc
from concourse.bass import ds, ts
from concourse.bass_utils import run_bass_kernel_spmd
from concourse.masks import make_identity

# problem shapes (hardcoded per contract)
B, T, K, M = 2, 2048, 4096, 12288
N = B * T
N_CORES = 8
M_CORE = M // N_CORES

MAGIC = 12582912.0  # 1.5 * 2**23: add+sub forces RNE rounding to integer

f32 = mybir.dt.float32
bf16 = mybir.dt.bfloat16
i32 = mybir.dt.int32
Alu = mybir.AluOpType
Act = mybir.ActivationFunctionType
Ax = mybir.AxisListType


def declare_io(nc: bass.Bass, n: int, k: int, m_core: int):
    x = nc.dram_tensor("x", [n, k], f32, kind="ExternalInput").ap()
    wp = nc.dram_tensor("wp", [m_core, k // 4], i32, kind="ExternalInput").ap()
    ws = nc.dram_tensor("ws", [1], f32, kind="ExternalInput").ap()
    out = nc.dram_tensor("out", [n, m_core], f32, kind="ExternalOutput").ap()
    return x, wp, ws, out


def emit(tc: tile.TileContext, ctx: ExitStack, aps, n: int, k: int, m_core: int):
    nc = tc.nc
    x, wp, ws, out = aps
    assert n % 128 == 0 and k % 512 == 0 and m_core % 128 == 0
    KT = k // 128  # number of 128-wide k tiles (== number of packed 32B blocks)
    NT = n // 128
    MC = 512 if m_core % 512 == 0 else m_core
    assert m_core % MC == 0
    NMC = m_core // MC  # m groups (moving-operand chunks)
    MTPG = MC // 128  # m tiles per group
    KH = KT // 2 if KT % 2 == 0 else KT  # ktiles per weight staging chunk
    NH = KT // KH

    const = ctx.enter_context(tc.tile_pool(name="const", bufs=1))
    ident = const.tile([128, 128], bf16)
    make_identity(nc, ident[:])
    wsb = const.tile([128, 1], f32)
    nc.gpsimd.dma_start(out=wsb[:], in_=ws.to_broadcast((128, 1)))

    # per-(m-group, k-tile) weight tiles: [128 k, MC m] bf16 codes {0..3}
    wt_pool = ctx.enter_context(tc.tile_pool(name="wt", bufs=1))
    WT = [
        [
            wt_pool.tile(
                [128, MC], bf16, tag=f"wt{mg}_{t}", name=f"wt{mg}_{t}"
            )
            for t in range(KT)
        ]
        for mg in range(NMC)
    ]

    # ---------------- x quantization (DVE) ----------------
    xf_pool = ctx.enter_context(tc.tile_pool(name="xf", bufs=2))
    xq_pool = ctx.enter_context(tc.tile_pool(name="xq", bufs=2))
    sm_pool = ctx.enter_context(tc.tile_pool(name="sm", bufs=3))
    qstate = {}
    xfs = {}

    def xdma(ni):
        # split across SWDGE queues so the first token tile lands sooner
        xf = xf_pool.tile([128, k], f32, tag="xf", name=f"xf{ni}")
        half = k // 2
        nc.gpsimd.dma_start(out=xf[:, 0:half], in_=x[ts(ni, 128), 0:half])
        nc.gpsimd.dma_start(out=xf[:, half:k], in_=x[ts(ni, 128), half:k])
        xfs[ni] = xf

    def quant(ni):
        if ni not in xfs:
            xdma(ni)
        xf = xfs.pop(ni)
        am = sm_pool.tile([128, 1], f32)
        nc.vector.tensor_reduce(
            am[:], xf[:], axis=Ax.X, op=Alu.max, apply_absolute_value=True
        )
        am2 = sm_pool.tile([128, 1], f32)
        nc.vector.tensor_scalar_max(am2[:], am[:], 1e-5)
        r = sm_pool.tile([128, 1], f32)  # 1/absmax (HW iterative divide)
        nc.vector.reciprocal(r[:], am2[:])
        q = sm_pool.tile([128, 1], f32)  # 127/absmax (to ~1ulp of reference)
        nc.vector.tensor_scalar_mul(q[:], r[:], 127.0)
        a = sm_pool.tile([128, 1], f32)  # act_scale = 1/q
        nc.vector.reciprocal(a[:], q[:])
        s = sm_pool.tile([128, 1], f32)  # act_scale * weight_scale
        nc.vector.tensor_tensor(s[:], a[:], wsb[:], op=Alu.mult)

        # x_i8 = RNE(x*q): in-place (x*q + MAGIC), then (- MAGIC) -> bf16;
        # accum_out gives rowsum(x_i8) in the same pass.
        nc.vector.tensor_scalar(
            out=xf[:], in0=xf[:], scalar1=q[:], scalar2=MAGIC,
            op0=Alu.mult, op1=Alu.add,
        )
        xq = xq_pool.tile([128, k], bf16)
        rs = sm_pool.tile([128, 1], f32)
        # out = in - MAGIC; accum_out = reduce_add(out) (op1 names the reduce)
        nc.vector.tensor_scalar(
            out=xq[:], in0=xf[:], scalar1=MAGIC, scalar2=None,
            op0=Alu.subtract, op1=Alu.add, accum_out=rs[:],
        )
        nrss = sm_pool.tile([128, 1], f32)  # -rowsum * s  (bias for dequant)
        nc.vector.tensor_scalar(
            out=nrss[:], in0=rs[:], scalar1=s[:], scalar2=-1.0,
            op0=Alu.mult, op1=Alu.mult,
        )
        qstate[ni] = (xq, s, nrss)

    # start the first x load now; its DVE compute is interleaved into the
    # weight prologue below (after the first chunk) so PE starts on W
    # transposes immediately while x data is still arriving.
    xdma(0)

    # ---------------- weight unpack prologue ----------------
    # packed byte b of row m (stored as one int32 each, value 0..255):
    # block = b//32, j = b%32; weight k = block*128 + g*32 + j uses shift 6-2g.
    pk_pool = ctx.enter_context(tc.tile_pool(name="pk", bufs=3))
    wi_pool = ctx.enter_context(tc.tile_pool(name="wi", bufs=2))
    wn_pool = ctx.enter_context(tc.tile_pool(name="wn", bufs=MTPG + 1))
    tpw_pool = ctx.enter_context(tc.tile_pool(name="tpw", bufs=2, space="PSUM"))
    kb_h = k // 4 // NH  # packed bytes per row per staging chunk

    TB = 4 if KH % 4 == 0 else 1  # ktiles per transpose/psum batch
    for mg in range(NMC):
        for h in range(NH):
            for j in range(MTPG):
                mi = mg * MTPG + j
                pkt = pk_pool.tile([128, kb_h], i32)
                nc.gpsimd.dma_start(
                    out=pkt[:], in_=wp[ts(mi, 128), ds(h * kb_h, kb_h)]
                )
                pk_v = pkt[:].rearrange("p (b j) -> p b j", j=32)
                # bitVec ops cannot cast: extract int32, then convert to bf16
                # (t-major: matmul stationary APs allow only one free dim)
                wn_i = wi_pool.tile([128, KH, 4, 32], i32)  # [m, kt, g, j]
                for g in range(4):
                    nc.vector.tensor_scalar(
                        out=wn_i[:, :, g, :],
                        in0=pk_v,
                        scalar1=6 - 2 * g,
                        scalar2=3,
                        op0=Alu.logical_shift_right,
                        op1=Alu.bitwise_and,
                    )
                wn = wn_pool.tile([128, KH, 4, 32], bf16, tag="wn")
                nc.vector.tensor_copy(
                    wn[:].rearrange("p t g j -> p (t g j)"),
                    wn_i[:].rearrange("p t g j -> p (t g j)"),
                )
                wn_v = wn[:].rearrange("p t g j -> p t (g j)")
                # per-mi transposes: 4 ktiles of this m-tile per PSUM bank
                for tq0 in range(0, KH, TB):
                    tp = tpw_pool.tile([128, TB * 128], bf16, tag="tpw")
                    for u in range(TB):
                        nc.tensor.transpose(
                            tp[:, ts(u, 128)], wn_v[:, tq0 + u, :], ident[:]
                        )
                    for u in range(TB):
                        t = h * KH + tq0 + u
                        nc.scalar.copy(
                            out=WT[mg][t][:, ts(j, 128)], in_=tp[:, ts(u, 128)]
                        )
            if mg == 0:
                # interleave the first two token tiles' quantize chains
                if h == 0 and 0 not in qstate:
                    xdma(1)
                    quant(0)
                if h == NH - 1 and 1 not in qstate:
                    quant(1)

    # ---------------- main loop over token tiles ----------------
    xt_pool = ctx.enter_context(tc.tile_pool(name="xt", bufs=2))
    ob_pool = ctx.enter_context(tc.tile_pool(name="ob", bufs=2))
    tpx_pool = ctx.enter_context(tc.tile_pool(name="tpx", bufs=2, space="PSUM"))
    mm_pool = ctx.enter_context(tc.tile_pool(name="mm", bufs=3, space="PSUM"))

    tstate = {}

    def xprep(ni):
        # transpose x_i8 -> xT [k%128, ktile, n%128]
        xq, _, _ = qstate[ni]
        xT = xt_pool.tile([128, KT, 128], bf16, tag="xT", name=f"xT{ni}")
        xq_v = xq[:].rearrange("p (t j) -> p t j", j=128)
        for tq in range((KT + 3) // 4):
            nt = min(4, KT - tq * 4)
            tp = tpx_pool.tile([128, 512], bf16, tag="tpx", name=f"tpx{ni}_{tq}")
            for j in range(nt):
                t = tq * 4 + j
                nc.tensor.transpose(tp[:, ts(j, 128)], xq_v[:, t, :], ident[:])
            nc.scalar.copy(
                out=xT[:, ds(tq * 4, nt), :],
                in_=tp[:, ds(0, nt * 128)].rearrange("p (a b) -> p a b", b=128),
            )
        tstate[ni] = xT

    # software pipeline: ni+1's quantize + transposes are emitted (and thus
    # statically scheduled on PE) BEFORE ni's matmuls, filling the xT-copy
    # latency bubble between a tile's transposes and its matmuls.
    xprep(0)
    for ni in range(NT):
        if ni + 2 < NT:
            quant(ni + 2)  # DVE chain for ni+2 runs during this step
        if ni + 1 < NT:
            if ni + 1 not in qstate:
                quant(ni + 1)
            xprep(ni + 1)  # xq(ni+1) was produced during the previous step
        xq, s, nrss = qstate.pop(ni)
        xT = tstate.pop(ni)

        for mc in range(NMC):
            ps = mm_pool.tile([128, MC], f32)
            for t in range(KT):
                nc.tensor.matmul(
                    ps[:],
                    lhsT=xT[:, t, :],
                    rhs=WT[mc][t][:],
                    start=(t == 0),
                    stop=(t == KT - 1),
                )
            ob = ob_pool.tile([128, MC], f32)
            # out = psum*s + (-rowsum*s)  == (psum - rowsum) * s
            nc.scalar.activation(
                ob[:], ps[:], Act.Identity, bias=nrss[:], scale=s[:]
            )
            nc.gpsimd.dma_start(out=out[ts(ni, 128), ds(mc * MC, MC)], in_=ob[:])


def build(n: int = N, k: int = K, m_core: int = M_CORE, num_devices: int = N_CORES):
    nc = bacc.Bacc(
        "TRN2", target_bir_lowering=False, debug=False, num_devices=num_devices
    )
    aps = declare_io(nc, n, k, m_core)
    with tile.TileContext(nc) as tc:
        with ExitStack() as ctx:
            emit(tc, ctx, aps, n, k, m_core)
    nc.compile()
    return nc


_CACHE: dict = {}


def kernel(x: np.ndarray, weight: np.ndarray, weight_scale: np.ndarray) -> np.ndarray:
    orig_shape = x.shape
    x2 = np.ascontiguousarray(x.reshape(-1, orig_shape[-1]), dtype=np.float32)
    w = np.ascontiguousarray(weight, dtype=np.int32)
    ws = np.ascontiguousarray(weight_scale, dtype=np.float32)
    assert x2.shape == (N, K) and w.shape == (M, K // 4)

    if "nc" not in _CACHE:
        _CACHE["nc"] = build()
    nc = _CACHE["nc"]

    in_maps = [
        {"x": x2, "wp": w[i * M_CORE : (i + 1) * M_CORE], "ws": ws}
        for i in range(N_CORES)
    ]
    res = run_bass_kernel_spmd(
        nc,
        in_maps,
        list(range(N_CORES)),
        trace=bool(int(os.environ.get("BITLINEAR_TRACE", "0"))),
    )
    _CACHE["last_result"] = res
    outs = [res.results[i]["out"] for i in range(N_CORES)]
    full = np.concatenate(outs, axis=1)
    return full.reshape(*orig_shape[:-1], M).astype(x.dtype, copy=False)


# revision 31
# speedup vs baseline: 1.0294x; 1.0099x over previous
"""BitLinear (2-bit ternary packed weights) Trainium2 Bass kernel.

Full-input contract: kernel(x, weight, weight_scale) -> (2, 2048, 12288) f32.
Tensor-parallel over 8 NeuronCores: weight rows (out_features) sharded
8 x 1536, x replicated, outputs concatenated host-side.

Math notes:
  - reference: x_i8 = round(x * 127/absmax_tok); W = unpack2bit(weight)-1
    out = (x_i8 @ W.T) * weight_scale * absmax_tok/127
  - we matmul against the raw 2-bit codes c in {0,1,2,3} (bf16-exact) and
    fold the -1 via out_int = psum - rowsum(x_i8); rowsum comes free via
    accum_out on the quantize op.
  - bf16 holds integers up to 256 exactly; f32 PSUM accumulation of
    integer products stays < 2^24 => matmul is bit-exact vs f32 reference.
  - rounding uses the +/- 1.5*2^23 magic trick == round-half-to-even
    (matches jnp.round). clip(-128,127) is a provable no-op since
    |x*q| <= 127*(1+eps).

Layout: weights are unpacked+transposed once into per-(m-group, k-tile)
SBUF tiles WT[mg][t] ([128 k, 512 m] bf16) so each matmul depends on
exactly one prologue producer (fine-grained Tile deps -> early PE start).
"""

import os
from contextlib import ExitStack

import numpy as np

import concourse.bass as bass
import concourse.mybir as mybir
import concourse.tile as tile
from concourse import bacc
from concourse.bass import ds, ts
from concourse.bass_utils import run_bass_kernel_spmd
from concourse.masks import make_identity

# problem shapes (hardcoded per contract)
B, T, K, M = 2, 2048, 4096, 12288
N = B * T
N_CORES = 8
M_CORE = M // N_CORES

MAGIC = 12582912.0  # 1.5 * 2**23: add+sub forces RNE rounding to integer

f32 = mybir.dt.float32
bf16 = mybir.dt.bfloat16
i32 = mybir.dt.int32
Alu = mybir.AluOpType
Act = mybir.ActivationFunctionType
Ax = mybir.AxisListType


def declare_io(nc: bass.Bass, n: int, k: int, m_core: int):
    x = nc.dram_tensor("x", [n, k], f32, kind="ExternalInput").ap()
    wp = nc.dram_tensor("wp", [m_core, k // 4], i32, kind="ExternalInput").ap()
    ws = nc.dram_tensor("ws", [1], f32, kind="ExternalInput").ap()
    out = nc.dram_tensor("out", [n, m_core], f32, kind="ExternalOutput").ap()
    return x, wp, ws, out


def emit(tc: tile.TileContext, ctx: ExitStack, aps, n: int, k: int, m_core: int):
    nc = tc.nc
    x, wp, ws, out = aps
    assert n % 128 == 0 and k % 512 == 0 and m_core % 128 == 0
    KT = k // 128  # number of 128-wide k tiles (== number of packed 32B blocks)
    NT = n // 128
    MC = 512 if m_core % 512 == 0 else m_core
    assert m_core % MC == 0
    NMC = m_core // MC  # m groups (moving-operand chunks)
    MTPG = MC // 128  # m tiles per group
    KH = KT // 2 if KT % 8 == 0 else KT  # ktiles per weight staging chunk
    NH = KT // KH

    const = ctx.enter_context(tc.tile_pool(name="const", bufs=1))
    ident = const.tile([128, 128], bf16)
    make_identity(nc, ident[:])
    wsb = const.tile([128, 1], f32)
    nc.gpsimd.dma_start(out=wsb[:], in_=ws.to_broadcast((128, 1)))

    # per-(m-group, k-tile) weight tiles: [128 k, MC m] bf16 codes {0..3}
    wt_pool = ctx.enter_context(tc.tile_pool(name="wt", bufs=1))
    WT = [
        [
            wt_pool.tile(
                [128, MC], bf16, tag=f"wt{mg}_{t}", name=f"wt{mg}_{t}"
            )
            for t in range(KT)
        ]
        for mg in range(NMC)
    ]

    # ---------------- x quantization (DVE) ----------------
    xf_pool = ctx.enter_context(tc.tile_pool(name="xf", bufs=2))
    xq_pool = ctx.enter_context(tc.tile_pool(name="xq", bufs=2))
    sm_pool = ctx.enter_context(tc.tile_pool(name="sm", bufs=3))
    qstate = {}
    xfs = {}

    def xdma(ni):
        # split across SWDGE queues so the first token tile lands sooner
        xf = xf_pool.tile([128, k], f32, tag="xf", name=f"xf{ni}")
        half = k // 2
        nc.gpsimd.dma_start(out=xf[:, 0:half], in_=x[ts(ni, 128), 0:half])
        nc.gpsimd.dma_start(out=xf[:, half:k], in_=x[ts(ni, 128), half:k])
        xfs[ni] = xf

    def quant(ni):
        if ni not in xfs:
            xdma(ni)
        xf = xfs.pop(ni)
        am = sm_pool.tile([128, 1], f32)
        nc.vector.tensor_reduce(
            am[:], xf[:], axis=Ax.X, op=Alu.max, apply_absolute_value=True
        )
        am2 = sm_pool.tile([128, 1], f32)
        nc.vector.tensor_scalar_max(am2[:], am[:], 1e-5)
        r = sm_pool.tile([128, 1], f32)  # 1/absmax (HW iterative divide)
        nc.vector.reciprocal(r[:], am2[:])
        q = sm_pool.tile([128, 1], f32)  # 127/absmax (to ~1ulp of reference)
        nc.vector.tensor_scalar_mul(q[:], r[:], 127.0)
        a = sm_pool.tile([128, 1], f32)  # act_scale = 1/q
        nc.vector.reciprocal(a[:], q[:])
        s = sm_pool.tile([128, 1], f32)  # act_scale * weight_scale
        nc.vector.tensor_tensor(s[:], a[:], wsb[:], op=Alu.mult)

        # x_i8 = RNE(x*q): in-place (x*q + MAGIC), then (- MAGIC) -> bf16;
        # accum_out gives rowsum(x_i8) in the same pass.
        nc.vector.tensor_scalar(
            out=xf[:], in0=xf[:], scalar1=q[:], scalar2=MAGIC,
            op0=Alu.mult, op1=Alu.add,
        )
        # xq is stored k-PERMUTED (g-major pseudo-k order, matching the
        # weight path): storage (g, t, j) <- natural k = (t, g, j). The
        # contraction is permutation-invariant, so matmuls see consistent k.
        xq = xq_pool.tile([128, 4, k // 128, 32], bf16, tag="xq", name=f"xq{ni}")
        rs = sm_pool.tile([128, 1], f32)
        # out = in - MAGIC; accum_out = reduce_add(out) (op1 names the reduce)
        nc.vector.tensor_scalar(
            out=xq[:].rearrange("p g t j -> p t g j"),
            in0=xf[:].rearrange("p (t g j) -> p t g j", g=4, j=32),
            scalar1=MAGIC, scalar2=None,
            op0=Alu.subtract, op1=Alu.add, accum_out=rs[:],
        )
        nrss = sm_pool.tile([128, 1], f32)  # -rowsum * s  (bias for dequant)
        nc.vector.tensor_scalar(
            out=nrss[:], in0=rs[:], scalar1=s[:], scalar2=-1.0,
            op0=Alu.mult, op1=Alu.mult,
        )
        qstate[ni] = (xq, s, nrss)

    # start the first x load now; its DVE compute is interleaved into the
    # weight prologue below (after the first chunk) so PE starts on W
    # transposes immediately while x data is still arriving.
    xdma(0)

    # ---------------- weight unpack prologue ----------------
    # packed byte b of row m (stored as one int32 each, value 0..255):
    # block = b//32, j = b%32; weight k = block*128 + g*32 + j uses shift 6-2g.
    pk_pool = ctx.enter_context(tc.tile_pool(name="pk", bufs=2))
    wi_pool = ctx.enter_context(tc.tile_pool(name="wi", bufs=2))
    wn_pool = ctx.enter_context(tc.tile_pool(name="wn", bufs=MTPG + 1))
    tpw_pool = ctx.enter_context(tc.tile_pool(name="tpw", bufs=2, space="PSUM"))
    kb_h = k // 4 // NH  # packed bytes per row per staging chunk

    assert KH % 4 == 0 and KT % 4 == 0
    for mg in range(NMC):
        for h in range(NH):
            wns = []
            for j in range(MTPG):
                mi = mg * MTPG + j
                pkt = pk_pool.tile([128, kb_h], i32)
                nc.gpsimd.dma_start(
                    out=pkt[:], in_=wp[ts(mi, 128), ds(h * kb_h, kb_h)]
                )
                pk_v = pkt[:].rearrange("p (b j) -> p b j", j=32)
                # bitVec ops cannot cast: extract int32 then convert to bf16.
                # g-major storage makes every extract write one contiguous
                # run (DVE accel) and every transpose read contiguous.
                wn_i = wi_pool.tile([128, 4, KH, 32], i32)  # [m, g, kt, j]
                for g in range(4):
                    nc.vector.tensor_scalar(
                        out=wn_i[:, g, :, :],
                        in0=pk_v,
                        scalar1=6 - 2 * g,
                        scalar2=3,
                        op0=Alu.logical_shift_right,
                        op1=Alu.bitwise_and,
                    )
                wn = wn_pool.tile([128, 4, KH, 32], bf16, tag="wn")
                nc.vector.tensor_copy(
                    wn[:].rearrange("p g t j -> p (g t j)"),
                    wn_i[:].rearrange("p g t j -> p (g t j)"),
                )
                wns.append(wn)
            # transpose pseudo-k-tiles: bank batches 4 m-tiles of one cx
            for g in range(4):
                for cc in range(KH // 4):
                    cx = g * (KT // 4) + h * (KH // 4) + cc
                    tp = tpw_pool.tile([128, MC], bf16, tag="tpw")
                    for j in range(MTPG):
                        wnf = wns[j][:].rearrange("p g t j -> p (g t j)")
                        nc.tensor.transpose(
                            tp[:, ts(j, 128)],
                            wnf[:, ds((g * KH + 4 * cc) * 32, 128)],
                            ident[:],
                        )
                    nc.scalar.copy(out=WT[mg][cx][:], in_=tp[:])
            if mg == 0:
                # interleave the first two token tiles' quantize chains
                if h == 0 and 0 not in qstate:
                    xdma(1)
                    quant(0)
                if h == NH - 1 and 1 not in qstate:
                    quant(1)

    # ---------------- main loop over token tiles ----------------
    xt_pool = ctx.enter_context(tc.tile_pool(name="xt", bufs=2))
    ob_pool = ctx.enter_context(tc.tile_pool(name="ob", bufs=2))
    tpx_pool = ctx.enter_context(tc.tile_pool(name="tpx", bufs=2, space="PSUM"))
    mm_pool = ctx.enter_context(tc.tile_pool(name="mm", bufs=3, space="PSUM"))

    tstate = {}

    def xprep(ni):
        # transpose x_i8 -> xT [k%128, ktile, n%128]
        xq, _, _ = qstate[ni]
        xT = xt_pool.tile([128, KT, 128], bf16, tag="xT", name=f"xT{ni}")
        xq_f = xq[:].rearrange("p g t j -> p (g t j)")
        for tq in range((KT + 3) // 4):
            nt = min(4, KT - tq * 4)
            tp = tpx_pool.tile([128, 512], bf16, tag="tpx", name=f"tpx{ni}_{tq}")
            for j in range(nt):
                t = tq * 4 + j
                nc.tensor.transpose(tp[:, ts(j, 128)], xq_f[:, ts(t, 128)], ident[:])
            nc.scalar.copy(
                out=xT[:, ds(tq * 4, nt), :],
                in_=tp[:, ds(0, nt * 128)].rearrange("p (a b) -> p a b", b=128),
            )
        tstate[ni] = xT

    # software pipeline: ni+1's quantize + transposes are emitted (and thus
    # statically scheduled on PE) BEFORE ni's matmuls, filling the xT-copy
    # latency bubble between a tile's transposes and its matmuls.
    xprep(0)
    for ni in range(NT):
        if ni + 2 < NT:
            quant(ni + 2)  # DVE chain for ni+2 runs during this step
        if ni + 1 < NT:
            if ni + 1 not in qstate:
                quant(ni + 1)
            xprep(ni + 1)  # xq(ni+1) was produced during the previous step
        xq, s, nrss = qstate.pop(ni)
        xT = tstate.pop(ni)

        for mc in range(NMC):
            ps = mm_pool.tile([128, MC], f32)
            for t in range(KT):
                nc.tensor.matmul(
                    ps[:],
                    lhsT=xT[:, t, :],
                    rhs=WT[mc][t][:],
                    start=(t == 0),
                    stop=(t == KT - 1),
                )
            ob = ob_pool.tile([128, MC], f32)
            # out = psum*s + (-rowsum*s)  == (psum - rowsum) * s
            nc.scalar.activation(
                ob[:], ps[:], Act.Identity, bias=nrss[:], scale=s[:]
            )
            nc.gpsimd.dma_start(out=out[ts(ni, 128), ds(mc * MC, MC)], in_=ob[:])


def build(n: int = N, k: int = K, m_core: int = M_CORE, num_devices: int = N_CORES):
    nc = bacc.Bacc(
        "TRN2", target_bir_lowering=False, debug=False, num_devices=num_devices
    )
    aps = declare_io(nc, n, k, m_core)
    with tile.TileContext(nc) as tc:
        with ExitStack() as ctx:
            emit(tc, ctx, aps, n, k, m_core)
    nc.compile()
    return nc


_CACHE: dict = {}


def kernel(x: np.ndarray, weight: np.ndarray, weight_scale: np.ndarray) -> np.ndarray:
    orig_shape = x.shape
    x2 = np.ascontiguousarray(x.reshape(-1, orig_shape[-1]), dtype=np.float32)
    w = np.ascontiguousarray(weight, dtype=np.int32)
    ws = np.ascontiguousarray(weight_scale, dtype=np.float32)
    assert x2.shape == (N, K) and w.shape == (M, K // 4)

    if "nc" not in _CACHE:
        _CACHE["nc"] = build()
    nc = _CACHE["nc"]

    in_maps = [
        {"x": x2, "wp": w[i * M_CORE : (i + 1) * M_CORE], "ws": ws}
        for i in range(N_CORES)
    ]
    res = run_bass_kernel_spmd(
        nc,
        in_maps,
        list(range(N_CORES)),
        trace=bool(int(os.environ.get("BITLINEAR_TRACE", "0"))),
    )
    _CACHE["last_result"] = res
    outs = [res.results[i]["out"] for i in range(N_CORES)]
    full = np.concatenate(outs, axis=1)
    return full.reshape(*orig_shape[:-1], M).astype(x.dtype, copy=False)


# revision 33
# speedup vs baseline: 1.0540x; 1.0239x over previous
"""BitLinear (2-bit ternary packed weights) Trainium2 Bass kernel.

Full-input contract: kernel(x, weight, weight_scale) -> (2, 2048, 12288) f32.
Tensor-parallel over 8 NeuronCores: weight rows (out_features) sharded
8 x 1536, x replicated, outputs concatenated host-side.

Math notes:
  - reference: x_i8 = round(x * 127/absmax_tok); W = unpack2bit(weight)-1
    out = (x_i8 @ W.T) * weight_scale * absmax_tok/127
  - we matmul against the raw 2-bit codes c in {0,1,2,3} (bf16-exact) and
    fold the -1 via out_int = psum - rowsum(x_i8); rowsum comes free via
    accum_out on the quantize op.
  - bf16 holds integers up to 256 exactly; f32 PSUM accumulation of
    integer products stays < 2^24 => matmul is bit-exact vs f32 reference.
  - rounding uses the +/- 1.5*2^23 magic trick == round-half-to-even
    (matches jnp.round). clip(-128,127) is a provable no-op since
    |x*q| <= 127*(1+eps).

Layout: weights are unpacked+transposed once into per-(m-group, k-tile)
SBUF tiles WT[mg][t] ([128 k, 512 m] bf16) so each matmul depends on
exactly one prologue producer (fine-grained Tile deps -> early PE start).
"""

import os
from contextlib import ExitStack

import numpy as np

import concourse.bass as bass
import concourse.mybir as mybir
import concourse.tile as tile
from concourse import bacc
from concourse.bass import ds, ts
from concourse.bass_utils import run_bass_kernel_spmd
from concourse.masks import make_identity

# problem shapes (hardcoded per contract)
B, T, K, M = 2, 2048, 4096, 12288
N = B * T
N_CORES = 8
M_CORE = M // N_CORES

MAGIC = 12582912.0  # 1.5 * 2**23: add+sub forces RNE rounding to integer

f32 = mybir.dt.float32
bf16 = mybir.dt.bfloat16
i32 = mybir.dt.int32
Alu = mybir.AluOpType
Act = mybir.ActivationFunctionType
Ax = mybir.AxisListType


def declare_io(nc: bass.Bass, n: int, k: int, m_core: int):
    x = nc.dram_tensor("x", [n, k], f32, kind="ExternalInput").ap()
    wp = nc.dram_tensor("wp", [m_core, k // 4], i32, kind="ExternalInput").ap()
    ws = nc.dram_tensor("ws", [1], f32, kind="ExternalInput").ap()
    out = nc.dram_tensor("out", [n, m_core], f32, kind="ExternalOutput").ap()
    return x, wp, ws, out


def emit(tc: tile.TileContext, ctx: ExitStack, aps, n: int, k: int, m_core: int):
    nc = tc.nc
    x, wp, ws, out = aps
    assert n % 128 == 0 and k % 512 == 0 and m_core % 128 == 0
    KT = k // 128  # number of 128-wide k tiles (== number of packed 32B blocks)
    NT = n // 128
    MC = 512 if m_core % 512 == 0 else m_core
    assert m_core % MC == 0
    NMC = m_core // MC  # m groups (moving-operand chunks)
    MTPG = MC // 128  # m tiles per group
    KH = KT // 2 if KT % 8 == 0 else KT  # ktiles per weight staging chunk
    NH = KT // KH

    const = ctx.enter_context(tc.tile_pool(name="const", bufs=1))
    ident = const.tile([128, 128], bf16)
    make_identity(nc, ident[:])
    wsb = const.tile([128, 1], f32)
    nc.gpsimd.dma_start(out=wsb[:], in_=ws.to_broadcast((128, 1)))

    # per-(m-group, k-tile) weight tiles: [128 k, MC m] bf16 codes {0..3}
    wt_pool = ctx.enter_context(tc.tile_pool(name="wt", bufs=1))
    WT = [
        [
            wt_pool.tile(
                [128, MC], bf16, tag=f"wt{mg}_{t}", name=f"wt{mg}_{t}"
            )
            for t in range(KT)
        ]
        for mg in range(NMC)
    ]

    # ---------------- x quantization (DVE) ----------------
    xf_pool = ctx.enter_context(tc.tile_pool(name="xf", bufs=2))
    xq_pool = ctx.enter_context(tc.tile_pool(name="xq", bufs=2))
    sm_pool = ctx.enter_context(tc.tile_pool(name="sm", bufs=3))
    qstate = {}
    xfs = {}

    def xdma(ni):
        # split across SWDGE queues so the first token tile lands sooner
        xf = xf_pool.tile([128, k], f32, tag="xf", name=f"xf{ni}")
        half = k // 2
        nc.gpsimd.dma_start(out=xf[:, 0:half], in_=x[ts(ni, 128), 0:half])
        nc.gpsimd.dma_start(out=xf[:, half:k], in_=x[ts(ni, 128), half:k])
        xfs[ni] = xf

    def quant(ni):
        if ni not in xfs:
            xdma(ni)
        xf = xfs.pop(ni)
        am = sm_pool.tile([128, 1], f32)
        nc.vector.tensor_reduce(
            am[:], xf[:], axis=Ax.X, op=Alu.max, apply_absolute_value=True
        )
        am2 = sm_pool.tile([128, 1], f32)
        nc.vector.tensor_scalar_max(am2[:], am[:], 1e-5)
        r = sm_pool.tile([128, 1], f32)  # 1/absmax (HW iterative divide)
        nc.vector.reciprocal(r[:], am2[:])
        q = sm_pool.tile([128, 1], f32)  # 127/absmax (to ~1ulp of reference)
        nc.vector.tensor_scalar_mul(q[:], r[:], 127.0)
        a = sm_pool.tile([128, 1], f32)  # act_scale = 1/q
        nc.vector.reciprocal(a[:], q[:])
        s = sm_pool.tile([128, 1], f32)  # act_scale * weight_scale
        nc.vector.tensor_tensor(s[:], a[:], wsb[:], op=Alu.mult)

        # x_i8 = RNE(x*q): in-place (x*q + MAGIC), then (- MAGIC) -> bf16;
        # accum_out gives rowsum(x_i8) in the same pass.
        nc.vector.tensor_scalar(
            out=xf[:], in0=xf[:], scalar1=q[:], scalar2=MAGIC,
            op0=Alu.mult, op1=Alu.add,
        )
        # xq is stored k-PERMUTED (g-major pseudo-k order, matching the
        # weight path): storage (g, t, j) <- natural k = (t, g, j). The
        # contraction is permutation-invariant, so matmuls see consistent k.
        xq = xq_pool.tile([128, 4, k // 128, 32], bf16, tag="xq", name=f"xq{ni}")
        rs = sm_pool.tile([128, 1], f32)
        # out = in - MAGIC; accum_out = reduce_add(out) (op1 names the reduce)
        nc.vector.tensor_scalar(
            out=xq[:].rearrange("p g t j -> p t g j"),
            in0=xf[:].rearrange("p (t g j) -> p t g j", g=4, j=32),
            scalar1=MAGIC, scalar2=None,
            op0=Alu.subtract, op1=Alu.add, accum_out=rs[:],
        )
        nrss = sm_pool.tile([128, 1], f32)  # -rowsum * s  (bias for dequant)
        nc.vector.tensor_scalar(
            out=nrss[:], in0=rs[:], scalar1=s[:], scalar2=-1.0,
            op0=Alu.mult, op1=Alu.mult,
        )
        qstate[ni] = (xq, s, nrss)

    # ---------------- weight unpack prologue ----------------
    # packed byte b of row m (stored as one int32 each, value 0..255):
    # block = b//32, j = b%32; weight k = block*128 + g*32 + j uses shift 6-2g.
    pk_pool = ctx.enter_context(tc.tile_pool(name="pk", bufs=2))
    wi_pool = ctx.enter_context(tc.tile_pool(name="wi", bufs=2))
    wn_pool = ctx.enter_context(tc.tile_pool(name="wn", bufs=MTPG + 1))
    tpw_pool = ctx.enter_context(tc.tile_pool(name="tpw", bufs=2, space="PSUM"))
    kb_h = k // 4 // NH  # packed bytes per row per staging chunk

    assert KH % 4 == 0 and KT % 4 == 0
    for mg in range(NMC):
        for h in range(NH):
            wns = []
            for j in range(MTPG):
                mi = mg * MTPG + j
                pkt = pk_pool.tile([128, kb_h], i32)
                nc.gpsimd.dma_start(
                    out=pkt[:], in_=wp[ts(mi, 128), ds(h * kb_h, kb_h)]
                )
                pk_v = pkt[:].rearrange("p (b j) -> p b j", j=32)
                # bitVec ops cannot cast: extract int32 then convert to bf16.
                # g-major storage makes every extract write one contiguous
                # run (DVE accel) and every transpose read contiguous.
                wn_i = wi_pool.tile([128, 4, KH, 32], i32)  # [m, g, kt, j]
                for g in range(4):
                    nc.vector.tensor_scalar(
                        out=wn_i[:, g, :, :],
                        in0=pk_v,
                        scalar1=6 - 2 * g,
                        scalar2=3,
                        op0=Alu.logical_shift_right,
                        op1=Alu.bitwise_and,
                    )
                wn = wn_pool.tile([128, 4, KH, 32], bf16, tag="wn")
                nc.vector.tensor_copy(
                    wn[:].rearrange("p g t j -> p (g t j)"),
                    wn_i[:].rearrange("p g t j -> p (g t j)"),
                )
                wns.append(wn)
                if mg == 0 and h == 0 and j == 0:
                    # first x load: after the first weight DMA (so PE's
                    # first transposes aren't starved behind it), but with
                    # enough lead before quant(0)'s DVE chain needs it
                    xdma(0)
            # transpose pseudo-k-tiles: bank batches 4 m-tiles of one cx
            for g in range(4):
                for cc in range(KH // 4):
                    cx = g * (KT // 4) + h * (KH // 4) + cc
                    tp = tpw_pool.tile([128, MC], bf16, tag="tpw")
                    for j in range(MTPG):
                        wnf = wns[j][:].rearrange("p g t j -> p (g t j)")
                        nc.tensor.transpose(
                            tp[:, ts(j, 128)],
                            wnf[:, ds((g * KH + 4 * cc) * 32, 128)],
                            ident[:],
                        )
                    nc.scalar.copy(out=WT[mg][cx][:], in_=tp[:])
            if mg == 0:
                # interleave the first two token tiles' quantize chains
                if h == 0 and 0 not in qstate:
                    xdma(1)
                    quant(0)
                if h == NH - 1 and 1 not in qstate:
                    quant(1)

    # ---------------- main loop over token tiles ----------------
    xt_pool = ctx.enter_context(tc.tile_pool(name="xt", bufs=2))
    ob_pool = ctx.enter_context(tc.tile_pool(name="ob", bufs=2))
    tpx_pool = ctx.enter_context(tc.tile_pool(name="tpx", bufs=2, space="PSUM"))
    mm_pool = ctx.enter_context(tc.tile_pool(name="mm", bufs=3, space="PSUM"))

    tstate = {}

    def xprep(ni):
        # transpose x_i8 -> xT [k%128, ktile, n%128]
        xq, _, _ = qstate[ni]
        xT = xt_pool.tile([128, KT, 128], bf16, tag="xT", name=f"xT{ni}")
        xq_f = xq[:].rearrange("p g t j -> p (g t j)")
        for tq in range((KT + 3) // 4):
            nt = min(4, KT - tq * 4)
            tp = tpx_pool.tile([128, 512], bf16, tag="tpx", name=f"tpx{ni}_{tq}")
            for j in range(nt):
                t = tq * 4 + j
                nc.tensor.transpose(tp[:, ts(j, 128)], xq_f[:, ts(t, 128)], ident[:])
            nc.scalar.copy(
                out=xT[:, ds(tq * 4, nt), :],
                in_=tp[:, ds(0, nt * 128)].rearrange("p (a b) -> p a b", b=128),
            )
        tstate[ni] = xT

    # software pipeline: ni+1's quantize + transposes are emitted (and thus
    # statically scheduled on PE) BEFORE ni's matmuls, filling the xT-copy
    # latency bubble between a tile's transposes and its matmuls.
    xprep(0)
    for ni in range(NT):
        if ni + 2 < NT:
            quant(ni + 2)  # DVE chain for ni+2 runs during this step
        if ni + 1 < NT:
            if ni + 1 not in qstate:
                quant(ni + 1)
            xprep(ni + 1)  # xq(ni+1) was produced during the previous step
        xq, s, nrss = qstate.pop(ni)
        xT = tstate.pop(ni)

        for mc in range(NMC):
            ps = mm_pool.tile([128, MC], f32)
            for t in range(KT):
                nc.tensor.matmul(
                    ps[:],
                    lhsT=xT[:, t, :],
                    rhs=WT[mc][t][:],
                    start=(t == 0),
                    stop=(t == KT - 1),
                )
            ob = ob_pool.tile([128, MC], f32)
            # out = psum*s + (-rowsum*s)  == (psum - rowsum) * s
            nc.scalar.activation(
                ob[:], ps[:], Act.Identity, bias=nrss[:], scale=s[:]
            )
            nc.gpsimd.dma_start(out=out[ts(ni, 128), ds(mc * MC, MC)], in_=ob[:])


def build(n: int = N, k: int = K, m_core: int = M_CORE, num_devices: int = N_CORES):
    nc = bacc.Bacc(
        "TRN2", target_bir_lowering=False, debug=False, num_devices=num_devices
    )
    aps = declare_io(nc, n, k, m_core)
    with tile.TileContext(nc) as tc:
        with ExitStack() as ctx:
            emit(tc, ctx, aps, n, k, m_core)
    nc.compile()
    return nc


_CACHE: dict = {}


def kernel(x: np.ndarray, weight: np.ndarray, weight_scale: np.ndarray) -> np.ndarray:
    orig_shape = x.shape
    x2 = np.ascontiguousarray(x.reshape(-1, orig_shape[-1]), dtype=np.float32)
    w = np.ascontiguousarray(weight, dtype=np.int32)
    ws = np.ascontiguousarray(weight_scale, dtype=np.float32)
    assert x2.shape == (N, K) and w.shape == (M, K // 4)

    if "nc" not in _CACHE:
        _CACHE["nc"] = build()
    nc = _CACHE["nc"]

    in_maps = [
        {"x": x2, "wp": w[i * M_CORE : (i + 1) * M_CORE], "ws": ws}
        for i in range(N_CORES)
    ]
    res = run_bass_kernel_spmd(
        nc,
        in_maps,
        list(range(N_CORES)),
        trace=bool(int(os.environ.get("BITLINEAR_TRACE", "0"))),
    )
    _CACHE["last_result"] = res
    outs = [res.results[i]["out"] for i in range(N_CORES)]
    full = np.concatenate(outs, axis=1)
    return full.reshape(*orig_shape[:-1], M).astype(x.dtype, copy=False)
